# revision 36
# baseline (speedup 1.0000x reference)
"""Causal self-attention (B=2, T=2048, C=1024, H=16) on 8 TRN2 NeuronCores.

Sharding: core = b*4 + hg  (data parallel over batch, tensor parallel over
4 head-groups of 4 heads). Each core computes its head-group's attention and
a partial output projection; the host sums the 4 partials per batch and adds
b_proj.

Per-core device program (v4 - PE-lean, pipeline-tight):
  - x, Wqk, Wv, Wp in f16; q/k/p/v/yT f16; all PSUM accumulation fp32.
  - xT loads in 512-column chunks, chunk-major, so qkv compute starts as
    soon as the first query window of every row block has landed.
  - v_aug is 65 cols per head [v(64) | 1]: the AV matmul lands v on PSUM
    rows 0..63 and accumulates the softmax denominator D on row 64 for free.
    Odd heads' rows are shifted to yT rows 64..127 by an SBUF->SBUF DMA
    (matmul outputs may not cross partition 64 unless based at 0, and
    compute engines are lane-locked, so DMA does the partition remap).
  - Attention runs per (head pair, 1024-wide query chunk), the two heads
    interleaved and software-pipelined one key-block stage: scores(jt) for
    both heads are emitted before the AV matmuls of jt-1, so every AV's exp
    input is long since ready and PE streams without dependency bubbles.
    Causality at 128-key-block granularity; ACT computes exp(s/32) straight
    out of PSUM; triangular masks multiply on GPSIMD. Narrow blocks
    (width <= 512) pack both heads into one PSUM tile and a single strided
    exp, halving ACT instruction overhead where per-op cost dominates.
  - o accumulates in per-head [128,512] half-bank tiles; each half is
    extracted (copy to yT + DVE reciprocal of the D row) as soon as its last
    AV lands, freeing PSUM early.
  - 1/D normalization: the two recip rows round-trip through DRAM and come
    back as one [128,1024] broadcast tile (0-stride source AP: even head ->
    partitions 0..63, odd -> 64..127); a single full-width DVE mul then
    normalizes both heads' yT rows in place. No PE broadcast matmuls.
  - PSUM: pool A = 2x[128,1024] (qkv/v/scores), pool B = 4x[128,512]
    (o accumulators; reused by the projection) = exactly 8 banks, so the
    next rep's qkv can claim pool A while the projection drains pool B.
  - Projection output is copied to SBUF as f16 (DVE/ACT alternating) and
    DMA'd out on two queues; host sums the f16 partials in f32.
"""

import math

import numpy as np

import concourse.bass as bass
import concourse.bacc as bacc
import concourse.mybir as mybir
from concourse import tile
from concourse.bass_utils import run_bass_kernel_spmd

B, T, C, H = 2, 2048, 1024, 16
HD = C // H   # 64
HPG = 4       # heads per group
NG = 4        # head groups
NCORES = 8

F32 = mybir.dt.float32
F16 = mybir.dt.float16
AF = mybir.ActivationFunctionType
SCALE = 1.0 / math.sqrt(C)  # 1/32

VW = 4 * 65  # 260: per head [v(64) | one]; AV out rows 0..64, D@64


def _pieces(L):
    """Bank-aligned (offset, width) pieces covering cols [L, 1024) of a
    1024-wide fp32 PSUM span (bank boundary at 512 fp32)."""
    assert 0 <= L < 1024 and L % 128 == 0
    if L < 512:
        return [(L, 512 - L), (512, 512)]
    return [(L, 1024 - L)]


def build_program(reps=1):
    nc = bacc.Bacc()

    xT = nc.dram_tensor("xT", [C, T], F16, kind="ExternalInput")
    wqk = nc.dram_tensor("wqk", [C, 512], F16, kind="ExternalInput")
    bqk = nc.dram_tensor("bqk", [128, 4], F32, kind="ExternalInput")
    wv = nc.dram_tensor("wv", [C, VW], F16, kind="ExternalInput")
    bv = nc.dram_tensor("bv", [1, VW], F16, kind="ExternalInput")
    wp = nc.dram_tensor("wp", [256, 1024], F16, kind="ExternalInput")
    mask = nc.dram_tensor("mask", [128, 128], F16, kind="ExternalInput")
    ones = nc.dram_tensor("ones", [1, 128], F16, kind="ExternalInput")
    out = nc.dram_tensor("out", [T, C], F16, kind="ExternalOutput")
    # 1/D round-trip scratch, one per (pair, ic): row 0 = even head's recip,
    # row 1 = odd head's. Read back with a 0-stride AP to broadcast across
    # partitions (rows 0..63 <- even, 64..127 <- odd).
    rbd = [[nc.dram_tensor(f"rbd{p}{i}", [2, 1024], F32, kind="Internal")
            for i in range(2)] for p in range(2)]

    with tile.TileContext(nc) as tc:
        with (
            tc.tile_pool(name="big", bufs=32) as big_pool,
            tc.tile_pool(name="pp", bufs=6) as p_pool,
            tc.tile_pool(name="osb", bufs=3) as o_pool,
            tc.tile_pool(name="wqk", bufs=8) as wqk_pool,
            tc.tile_pool(name="wv", bufs=8) as wv_pool,
            tc.tile_pool(name="qkT", bufs=4) as qkT_pool,
            tc.tile_pool(name="vsb", bufs=16) as v_pool,
            tc.tile_pool(name="yT", bufs=2) as yT_pool,
            tc.tile_pool(name="wp", bufs=2) as wp_pool,
            tc.tile_pool(name="rb", bufs=2) as rb_pool,
            tc.tile_pool(name="consts", bufs=1) as c_pool,
            tc.tile_pool(name="psA", bufs=2, space="PSUM") as psA,
            tc.tile_pool(name="psB", bufs=4, space="PSUM") as psB,
        ):
          for rep in range(reps):
            # ---- loads (wqk/xt interleaved so compute starts early) ----
            rw = c_pool.tile([128, T], F32, tag="rw")
            rwo = c_pool.tile([128, T], F32, tag="rwo")
            stage = c_pool.tile([64, T], F16, tag="stage")
            xt_sb, wqk_sb, wv_sb = [], [], []
            # spread load DMAs over the DGE paths so issue doesn't serialize
            for ct in range(8):
                w_ = wqk_pool.tile([128, 512], F16, tag="wqk", name=f"wqk{ct}")
                weng = nc.scalar if ct % 2 == 0 else nc.sync
                weng.dma_start(w_[:], wqk[ct * 128:(ct + 1) * 128, :])
                wqk_sb.append(w_)
                xt_sb.append([
                    big_pool.tile([128, 512], F16, tag="big",
                                  name=f"xt{ct}_{chk}")
                    for chk in range(4)
                ])
            # chunk-major load order: the first 512 query columns of every
            # row-block land first, so qkv compute starts early.
            for chk in range(4):
                for ct in range(8):
                    eng = nc.sync if (ct + chk) % 2 == 0 else nc.scalar
                    eng.dma_start(
                        xt_sb[ct][chk][:],
                        xT[ct * 128:(ct + 1) * 128,
                           chk * 512:(chk + 1) * 512],
                    )
            for ct in range(8):
                t_ = wv_pool.tile([128, VW], F16, tag="wv", name=f"wv{ct}")
                nc.gpsimd.dma_start(t_[:], wv[ct * 128:(ct + 1) * 128, :])
                wv_sb.append(t_)
            bqk_sb = c_pool.tile([128, 4], F32, tag="bqk")
            nc.gpsimd.dma_start(bqk_sb[:], bqk[:])
            bv_sb = c_pool.tile([1, VW], F16, tag="bv")
            nc.gpsimd.dma_start(bv_sb[:], bv[:])
            mask_sb = c_pool.tile([128, 128], F16, tag="mask")
            nc.gpsimd.dma_start(mask_sb[:], mask[:])
            ones_sb = c_pool.tile([1, 128], F16, tag="ones")
            nc.gpsimd.dma_start(ones_sb[:], ones[:])
            wp_sb = []
            for mt in range(2):
                t_ = wp_pool.tile([128, 1024], F16, tag="wp", name=f"wp{mt}")
                nc.sync.dma_start(t_[:], wp[mt * 128:(mt + 1) * 128, :])
                wp_sb.append(t_)

            qkT_sb = [
                qkT_pool.tile([128, T], F16, tag="qkT", name=f"qkT{j}")
                for j in range(4)
            ]
            yT_sb = [
                yT_pool.tile([128, T], F16, tag="yT", name=f"yT{m}")
                for m in range(2)
            ]

            def emit_qk(jts):
                for ch in range(4):
                    for jt in jts:
                        ps = psA.tile([128, 1024], F32, tag="A", name="qk_ps")
                        for ct in range(8):
                            nc.tensor.matmul(
                                ps[:, 0:512],
                                wqk_sb[ct][:, jt * 128:(jt + 1) * 128],
                                xt_sb[ct][ch][:],
                                start=(ct == 0),
                                stop=(ct == 7),
                            )
                        nc.vector.tensor_scalar_add(
                            qkT_sb[jt][:, ch * 512:(ch + 1) * 512],
                            ps[:, 0:512],
                            bqk_sb[:, jt:jt + 1],
                        )

            def emit_v():
                v_sb = []
                for tt in range(16):
                    ps = psA.tile([128, 1024], F32, tag="A", name="v_ps")
                    for ct in range(8):
                        nc.tensor.matmul(
                            ps[:, 0:VW],
                            xt_sb[ct][tt // 4][:, (tt % 4) * 128:
                                               (tt % 4 + 1) * 128],
                            wv_sb[ct][:, 0:VW],
                            start=(ct == 0),
                            stop=False,
                        )
                    nc.tensor.matmul(
                        ps[:, 0:VW],
                        ones_sb[0:1, 0:128],
                        bv_sb[0:1, 0:VW],
                        start=False,
                        stop=True,
                    )
                    t_ = v_pool.tile([128, VW], F16, tag="v", name=f"v{tt}")
                    nc.vector.tensor_copy(t_[:], ps[:, 0:VW])
                    v_sb.append(t_)
                return v_sb

            def emit_attention_pair(pair, ic, v_sb):
                """Both heads of a pair for query chunk ic, software-pipelined
                one jt stage: scores(jt) for h0,h1 are emitted before the AV
                matmuls of jt-1, so every AV's exp input is long since ready
                and PE streams without dependency bubbles. Each o half-bank is
                extracted (copy to yT + 1/D recip) as soon as it completes."""
                i0 = ic * 1024
                njt = 8 * (ic + 1)
                jtA_last = 8 * ic + 3
                geo = []
                o_halves = []
                for h2 in range(2):
                    # every head: [v(64)|one] -> out rows 0..64, D@64; the odd
                    # head's rows are DMA-shifted to yT rows 64..127 later.
                    geo.append(dict(
                        qrow=h2 * 64,
                        blk=(2 * pair + h2) * 65,
                        bw=65,
                        orow=0,
                    ))
                    o_halves.append([
                        psB.tile([128, 512], F32, tag="B", name=f"o{h2}_{half}")
                        for half in range(2)
                    ])
                q_tile = qkT_sb[pair]
                k_tile = qkT_sb[2 + pair]

                def extract_half(h2, half):
                    o_x = o_halves[h2][half]
                    c0 = i0 + half * 512
                    if h2 == 0:
                        nc.vector.tensor_copy(
                            yT_sb[pair][0:64, c0:c0 + 512], o_x[0:64, :]
                        )
                        nc.vector.reciprocal(
                            rw[64:65, c0:c0 + 512], o_x[64:65, :]
                        )
                    else:
                        # engines are lane-locked; DMA shifts partitions
                        # 0..63 -> yT rows 64..127.
                        nc.vector.tensor_copy(
                            stage[0:64, c0:c0 + 512], o_x[0:64, :]
                        )
                        nc.sync.dma_start(
                            yT_sb[pair][64:128, c0:c0 + 512],
                            stage[0:64, c0:c0 + 512],
                        )
                        nc.vector.reciprocal(
                            rwo[64:65, c0:c0 + 512], o_x[64:65, :]
                        )

                p_prev = None
                for jt in range(njt + 1):
                    p_cur = None
                    if jt < njt:
                        L = max(0, jt * 128 - i0)
                        w = 1024 - L
                        if w <= 512:
                            # narrow block: both heads share one psA tile
                            # (h0 in bank A at col 0, h1 in bank B at col 512)
                            # and a single strided exp covers both.
                            s_ps = psA.tile([128, 1024], F32, tag="A",
                                            name="s_ps")
                            for h2 in range(2):
                                g = geo[h2]
                                nc.tensor.matmul(
                                    s_ps[:, 512 * h2:512 * h2 + w],
                                    k_tile[g["qrow"]:g["qrow"] + 64,
                                           jt * 128:(jt + 1) * 128],
                                    q_tile[g["qrow"]:g["qrow"] + 64,
                                           i0 + L:i0 + L + w],
                                    start=True,
                                    stop=True,
                                )
                            p_sb = p_pool.tile([128, 1024], F16, tag="p",
                                               name="p_sb")
                            s3 = s_ps[:].rearrange("p (b c) -> p b c", b=2)
                            p3 = p_sb[:].rearrange("p (b c) -> p b c", b=2)
                            nc.scalar.activation(
                                p3[:, :, 0:w], s3[:, :, 0:w], AF.Exp,
                                scale=SCALE
                            )
                            if jt * 128 >= i0:  # diagonal: triangular mask
                                for h2 in range(2):
                                    nc.gpsimd.tensor_mul(
                                        p_sb[:, 512 * h2:512 * h2 + 128],
                                        p_sb[:, 512 * h2:512 * h2 + 128],
                                        mask_sb[:]
                                    )
                            p_cur = (p_sb, True)
                        else:
                            ps2 = []
                            for h2 in range(2):
                                g = geo[h2]
                                s_ps = psA.tile([128, 1024], F32, tag="A",
                                                name="s_ps")
                                for off, ww in _pieces(L):
                                    nc.tensor.matmul(
                                        s_ps[:, off:off + ww],
                                        k_tile[g["qrow"]:g["qrow"] + 64,
                                               jt * 128:(jt + 1) * 128],
                                        q_tile[g["qrow"]:g["qrow"] + 64,
                                               i0 + off:i0 + off + ww],
                                        start=True,
                                        stop=True,
                                    )
                                p_sb = p_pool.tile([128, 1024], F16, tag="p",
                                                   name="p_sb")
                                nc.scalar.activation(
                                    p_sb[:, L:1024], s_ps[:, L:1024], AF.Exp,
                                    scale=SCALE
                                )
                                if jt * 128 >= i0:  # diagonal: triangular mask
                                    nc.gpsimd.tensor_mul(
                                        p_sb[:, L:L + 128], p_sb[:, L:L + 128],
                                        mask_sb[:]
                                    )
                                ps2.append(p_sb)
                            p_cur = (ps2, False)
                    if jt > 0:
                        ja = jt - 1
                        La = max(0, ja * 128 - i0)
                        wa = 1024 - La
                        packed = p_prev[1]
                        for h2 in range(2):
                            g = geo[h2]
                            vap = v_sb[ja][:, g["blk"]:g["blk"] + g["bw"]]
                            if packed:
                                # p at col 512*h2, o cols [La-512, 512) bank B
                                nc.tensor.matmul(
                                    o_halves[h2][1][
                                        g["orow"]:g["orow"] + g["bw"],
                                        La - 512:512],
                                    vap,
                                    p_prev[0][:, 512 * h2:512 * h2 + wa],
                                    start=(ja == 0),
                                    stop=(ja == njt - 1),
                                    skip_group_check=True,
                                )
                            else:
                                for off, ww in _pieces(La):
                                    half = 0 if off < 512 else 1
                                    last = jtA_last if half == 0 else njt - 1
                                    nc.tensor.matmul(
                                        o_halves[h2][half][
                                            g["orow"]:g["orow"] + g["bw"],
                                            off - 512 * half:
                                            off - 512 * half + ww],
                                        vap,
                                        p_prev[0][h2][:, off:off + ww],
                                        start=(ja == 0),
                                        stop=(ja == last),
                                        skip_group_check=True,
                                    )
                            if ja == jtA_last:
                                extract_half(h2, 0)
                            if ja == njt - 1:
                                extract_half(h2, 1)
                    p_prev = p_cur

            def emit_normalize(pair, ic):
                """GPSIMD-broadcast the pair's 1/D rows across partitions
                (even head -> rows 0..63, odd -> 64..127, matching the yT row
                split), then one full-width mul normalizes both heads' yT."""
                i0 = ic * 1024
                sl = slice(i0, i0 + 1024)
                rb_sb = rb_pool.tile([128, 1024], F32, tag="rb", name="rb_sb")
                scr = rbd[pair][ic]
                nc.gpsimd.dma_start(scr[0:1, :], rw[64:65, sl])
                nc.gpsimd.dma_start(scr[1:2, :], rwo[64:65, sl])
                nc.gpsimd.dma_start(
                    rb_sb[:], scr[:, :].unsqueeze(1).broadcast_to((2, 64, 1024))
                )
                yt = yT_sb[pair]
                nc.vector.tensor_mul(yt[:, sl], yt[:, sl], rb_sb[:])

            def emit_proj(tts):
                # proj runs on the psB ring ([128,512] halves), which is free
                # after attention — so the next rep's qkv can claim psA
                # without waiting for proj to drain.
                for tt in tts:
                    o_sb = o_pool.tile([128, 1024], F16, tag="o", name="o_sb")
                    for nch in range(2):
                        ps = psB.tile([128, 512], F32, tag="B", name="pj_ps")
                        for mt in range(2):
                            nc.tensor.matmul(
                                ps[:, 0:512],
                                yT_sb[mt][:, tt * 128:(tt + 1) * 128],
                                wp_sb[mt][:, nch * 512:(nch + 1) * 512],
                                start=(mt == 0),
                                stop=(mt == 1),
                            )
                        if (tt + nch) % 2 == 0:
                            nc.vector.tensor_copy(
                                o_sb[:, nch * 512:(nch + 1) * 512], ps[:, 0:512]
                            )
                        else:
                            nc.scalar.copy(
                                o_sb[:, nch * 512:(nch + 1) * 512], ps[:, 0:512]
                            )
                    eng = nc.sync if tt % 2 == 0 else nc.scalar
                    eng.dma_start(out[tt * 128:(tt + 1) * 128, :], o_sb[:])

            emit_qk((0, 2))        # q and k tiles for head pair A (h0, h1)
            v_sb = emit_v()
            emit_qk((1, 3))        # head pair B (h2, h3)
            for pair in range(2):
                for ic in range(2):
                    emit_attention_pair(pair, ic, v_sb)
                    emit_normalize(pair, ic)
            emit_proj(range(16))

    if not nc.is_finalized():
        nc.finalize()
    return nc


def host_prep(x, W_attn, b_attn, W_proj):
    f16 = np.float16
    x = np.ascontiguousarray(np.asarray(x, np.float32))
    W_attn = np.ascontiguousarray(np.asarray(W_attn, np.float32))
    b_attn = np.ascontiguousarray(np.asarray(b_attn, np.float32))
    W_proj = np.ascontiguousarray(np.asarray(W_proj, np.float32))
    mask = np.triu(np.ones((128, 128), f16))
    ones = np.ones((1, 128), f16)
    per_group = []
    for hg in range(NG):
        heads = [hg * HPG + i for i in range(HPG)]
        wq = np.concatenate([W_attn[:, h * HD:(h + 1) * HD] for h in heads], axis=1)
        wk = np.concatenate(
            [W_attn[:, C + h * HD:C + (h + 1) * HD] for h in heads], axis=1
        )
        wqk_ = np.ascontiguousarray(np.concatenate([wq, wk], axis=1).astype(f16))
        bq = np.concatenate([b_attn[h * HD:(h + 1) * HD] for h in heads])
        bk = np.concatenate([b_attn[C + h * HD:C + (h + 1) * HD] for h in heads])
        bqk_ = np.ascontiguousarray(np.concatenate([bq, bk]).reshape(4, 128).T)
        wv_ = np.zeros((C, VW), np.float32)
        bv_ = np.zeros((1, VW), np.float32)
        for i, h in enumerate(heads):
            v_off = i * 65
            one_off = i * 65 + 64
            wv_[:, v_off:v_off + 64] = \
                W_attn[:, 2 * C + h * HD:2 * C + (h + 1) * HD]
            bv_[0, v_off:v_off + 64] = \
                b_attn[2 * C + h * HD:2 * C + (h + 1) * HD]
            bv_[0, one_off] = 1.0
        wp_ = np.ascontiguousarray(
            np.concatenate(
                [W_proj[h * HD:(h + 1) * HD, :] for h in heads], axis=0
            ).astype(f16)
        )
        per_group.append((wqk_, bqk_, wv_.astype(f16), bv_.astype(f16), wp_))
    in_maps = []
    for b in range(B):
        xT_b = np.ascontiguousarray(x[b].T.astype(f16))
        for hg in range(NG):
            wqk_, bqk_, wv_, bv_, wp_ = per_group[hg]
            in_maps.append(
                dict(xT=xT_b, wqk=wqk_, bqk=bqk_, wv=wv_, bv=bv_, wp=wp_,
                     mask=mask, ones=ones)
            )
    return in_maps


_prog_cache = {}


def _get_program():
    if "nc" not in _prog_cache:
        _prog_cache["nc"] = build_program()
    return _prog_cache["nc"]


def run_cores(in_maps, trace=False, **kw):
    return run_bass_kernel_spmd(
        _get_program(), in_maps, list(range(NCORES)), trace=trace, **kw
    )


def kernel(x, W_attn, b_attn, W_proj, b_proj):
    in_maps = host_prep(x, W_attn, b_attn, W_proj)
    br = run_cores(in_maps)
    b_proj = np.asarray(b_proj, np.float32)
    y = np.zeros((B, T, C), np.float32)
    for b in range(B):
        acc = np.zeros((T, C), np.float32)
        for hg in range(NG):
            acc += np.asarray(br.results[b * NG + hg]["out"], np.float32)
        y[b] = acc + b_proj[None, :]
    return y


# revision 44
# speedup vs baseline: 1.0288x; 1.0288x over previous
"""Causal self-attention (B=2, T=2048, C=1024, H=16) on 8 TRN2 NeuronCores.

Sharding: core = b*4 + hg  (data parallel over batch, tensor parallel over
4 head-groups of 4 heads). Each core computes its head-group's attention and
a partial output projection; the host sums the 4 partials per batch and adds
b_proj.

Per-core device program (v4 - PE-lean, pipeline-tight):
  - x, Wqk, Wv, Wp in f16; q/k/p/v/yT f16; all PSUM accumulation fp32.
  - xT loads in 512-column chunks, chunk-major, so qkv compute starts as
    soon as the first query window of every row block has landed.
  - v_aug is 65 cols per head [v(64) | 1]: the AV matmul lands v on PSUM
    rows 0..63 and accumulates the softmax denominator D on row 64 for free.
    Odd heads' rows are shifted to yT rows 64..127 by an SBUF->SBUF DMA
    (matmul outputs may not cross partition 64 unless based at 0, and
    compute engines are lane-locked, so DMA does the partition remap).
  - Attention runs per (head pair, 1024-wide query chunk), the two heads
    interleaved and software-pipelined one key-block stage: scores(jt) for
    both heads are emitted before the AV matmuls of jt-1, so every AV's exp
    input is long since ready and PE streams without dependency bubbles.
    Causality at 128-key-block granularity; ACT computes exp(s/32) straight
    out of PSUM; triangular masks multiply on GPSIMD. Narrow blocks
    (width <= 512) pack both heads into one PSUM tile and a single strided
    exp, halving ACT instruction overhead where per-op cost dominates.
  - The attention phase is locally exp(ACT)-bound (77us of exp vs 58us of
    score+AV matmul), so later-needed independent matmul work is injected
    into the in-order PE stream inside the jt loops: v tiles 8..15 (only
    read by ic=1 key blocks) go into pair 0 / chunk 0, and pair 1's qk
    projections go into pair 0 / chunk 1. Injected PSUM->SBUF copies run on
    DVE to keep the ACT exp stream clean.
  - o accumulates in per-head [128,512] half-bank tiles; each half is
    extracted (copy to yT + DVE reciprocal of the D row) as soon as its last
    AV lands, freeing PSUM early.
  - 1/D normalization: the two recip rows round-trip through DRAM and come
    back as one [128,1024] broadcast tile (0-stride source AP: even head ->
    partitions 0..63, odd -> 64..127); a single full-width DVE mul then
    normalizes both heads' yT rows in place. No PE broadcast matmuls.
  - PSUM: pool A = 2x[128,1024] (qkv/v/scores), pool B = 4x[128,512]
    (o accumulators; reused by the projection) = exactly 8 banks, so the
    next rep's qkv can claim pool A while the projection drains pool B.
  - Projection output is copied to SBUF as f16 (DVE/ACT alternating) and
    DMA'd out on two queues; host sums the f16 partials in f32.
"""

import math

import numpy as np

import concourse.bass as bass
import concourse.bacc as bacc
import concourse.mybir as mybir
from concourse import tile
from concourse.bass_utils import run_bass_kernel_spmd

B, T, C, H = 2, 2048, 1024, 16
HD = C // H   # 64
HPG = 4       # heads per group
NG = 4        # head groups
NCORES = 8

F32 = mybir.dt.float32
F16 = mybir.dt.float16
AF = mybir.ActivationFunctionType
SCALE = 1.0 / math.sqrt(C)  # 1/32

VW = 4 * 65  # 260: per head [v(64) | one]; AV out rows 0..64, D@64


def _pieces(L):
    """Bank-aligned (offset, width) pieces covering cols [L, 1024) of a
    1024-wide fp32 PSUM span (bank boundary at 512 fp32)."""
    assert 0 <= L < 1024 and L % 128 == 0
    if L < 512:
        return [(L, 512 - L), (512, 512)]
    return [(L, 1024 - L)]


def build_program(reps=1):
    nc = bacc.Bacc()

    xT = nc.dram_tensor("xT", [C, T], F16, kind="ExternalInput")
    wqk = nc.dram_tensor("wqk", [C, 512], F16, kind="ExternalInput")
    bqk = nc.dram_tensor("bqk", [128, 4], F32, kind="ExternalInput")
    wv = nc.dram_tensor("wv", [C, VW], F16, kind="ExternalInput")
    bv = nc.dram_tensor("bv", [1, VW], F16, kind="ExternalInput")
    wp = nc.dram_tensor("wp", [256, 1024], F16, kind="ExternalInput")
    mask = nc.dram_tensor("mask", [128, 128], F16, kind="ExternalInput")
    ones = nc.dram_tensor("ones", [1, 128], F16, kind="ExternalInput")
    out = nc.dram_tensor("out", [T, C], F16, kind="ExternalOutput")
    # 1/D round-trip scratch, one per (pair, ic): row 0 = even head's recip,
    # row 1 = odd head's. Read back with a 0-stride AP to broadcast across
    # partitions (rows 0..63 <- even, 64..127 <- odd).
    rbd = [[nc.dram_tensor(f"rbd{p}{i}", [2, 1024], F32, kind="Internal")
            for i in range(2)] for p in range(2)]

    with tile.TileContext(nc) as tc:
        with (
            tc.tile_pool(name="big", bufs=32) as big_pool,
            tc.tile_pool(name="pp", bufs=6) as p_pool,
            tc.tile_pool(name="osb", bufs=3) as o_pool,
            tc.tile_pool(name="wqk", bufs=8) as wqk_pool,
            tc.tile_pool(name="wv", bufs=8) as wv_pool,
            tc.tile_pool(name="qkT", bufs=4) as qkT_pool,
            tc.tile_pool(name="vsb", bufs=16) as v_pool,
            tc.tile_pool(name="yT", bufs=2) as yT_pool,
            tc.tile_pool(name="wp", bufs=2) as wp_pool,
            tc.tile_pool(name="rb", bufs=2) as rb_pool,
            tc.tile_pool(name="consts", bufs=1) as c_pool,
            tc.tile_pool(name="psA", bufs=2, space="PSUM") as psA,
            tc.tile_pool(name="psB", bufs=4, space="PSUM") as psB,
        ):
          for rep in range(reps):
            # ---- loads (wqk/xt interleaved so compute starts early) ----
            rw = c_pool.tile([128, T], F32, tag="rw")
            rwo = c_pool.tile([128, T], F32, tag="rwo")
            stage = c_pool.tile([64, T], F16, tag="stage")
            xt_sb, wqk_sb, wv_sb = [], [], []
            # spread load DMAs over the DGE paths so issue doesn't serialize
            for ct in range(8):
                w_ = wqk_pool.tile([128, 512], F16, tag="wqk", name=f"wqk{ct}")
                weng = nc.scalar if ct % 2 == 0 else nc.sync
                weng.dma_start(w_[:], wqk[ct * 128:(ct + 1) * 128, :])
                wqk_sb.append(w_)
                xt_sb.append([
                    big_pool.tile([128, 512], F16, tag="big",
                                  name=f"xt{ct}_{chk}")
                    for chk in range(4)
                ])
            # chunk-major load order: the first 512 query columns of every
            # row-block land first, so qkv compute starts early.
            for chk in range(4):
                for ct in range(8):
                    eng = nc.sync if (ct + chk) % 2 == 0 else nc.scalar
                    eng.dma_start(
                        xt_sb[ct][chk][:],
                        xT[ct * 128:(ct + 1) * 128,
                           chk * 512:(chk + 1) * 512],
                    )
            for ct in range(8):
                t_ = wv_pool.tile([128, VW], F16, tag="wv", name=f"wv{ct}")
                nc.gpsimd.dma_start(t_[:], wv[ct * 128:(ct + 1) * 128, :])
                wv_sb.append(t_)
            bqk_sb = c_pool.tile([128, 4], F32, tag="bqk")
            nc.gpsimd.dma_start(bqk_sb[:], bqk[:])
            bv_sb = c_pool.tile([1, VW], F16, tag="bv")
            nc.gpsimd.dma_start(bv_sb[:], bv[:])
            mask_sb = c_pool.tile([128, 128], F16, tag="mask")
            nc.gpsimd.dma_start(mask_sb[:], mask[:])
            ones_sb = c_pool.tile([1, 128], F16, tag="ones")
            nc.gpsimd.dma_start(ones_sb[:], ones[:])
            wp_sb = []
            for mt in range(2):
                t_ = wp_pool.tile([128, 1024], F16, tag="wp", name=f"wp{mt}")
                nc.sync.dma_start(t_[:], wp[mt * 128:(mt + 1) * 128, :])
                wp_sb.append(t_)

            qkT_sb = [
                qkT_pool.tile([128, T], F16, tag="qkT", name=f"qkT{j}")
                for j in range(4)
            ]
            yT_sb = [
                yT_pool.tile([128, T], F16, tag="yT", name=f"yT{m}")
                for m in range(2)
            ]

            def qk_group(jt, ch):
                def go():
                    ps = psA.tile([128, 1024], F32, tag="A", name="qk_ps")
                    for ct in range(8):
                        nc.tensor.matmul(
                            ps[:, 0:512],
                            wqk_sb[ct][:, jt * 128:(jt + 1) * 128],
                            xt_sb[ct][ch][:],
                            start=(ct == 0),
                            stop=(ct == 7),
                        )
                    nc.vector.tensor_scalar_add(
                        qkT_sb[jt][:, ch * 512:(ch + 1) * 512],
                        ps[:, 0:512],
                        bqk_sb[:, jt:jt + 1],
                    )
                return go

            def emit_qk(jts):
                for ch in range(4):
                    for jt in jts:
                        qk_group(jt, ch)()

            v_sb = [
                v_pool.tile([128, VW], F16, tag="v", name=f"v{tt}")
                for tt in range(16)
            ]

            def v_group(tt, eng):
                def go():
                    ps = psA.tile([128, 1024], F32, tag="A", name="v_ps")
                    for ct in range(8):
                        nc.tensor.matmul(
                            ps[:, 0:VW],
                            xt_sb[ct][tt // 4][:, (tt % 4) * 128:
                                               (tt % 4 + 1) * 128],
                            wv_sb[ct][:, 0:VW],
                            start=(ct == 0),
                            stop=False,
                        )
                    nc.tensor.matmul(
                        ps[:, 0:VW],
                        ones_sb[0:1, 0:128],
                        bv_sb[0:1, 0:VW],
                        start=False,
                        stop=True,
                    )
                    eng(v_sb[tt][:], ps[:, 0:VW])
                return go

            def emit_attention_pair(pair, ic, v_sb, inject=(), inj_start=1):
                """Both heads of a pair for query chunk ic, software-pipelined
                one jt stage: scores(jt) for h0,h1 are emitted before the AV
                matmuls of jt-1, so every AV's exp input is long since ready
                and PE streams without dependency bubbles. Each o half-bank is
                extracted (copy to yT + 1/D recip) as soon as it completes."""
                i0 = ic * 1024
                njt = 8 * (ic + 1)
                jtA_last = 8 * ic + 3
                geo = []
                o_halves = []
                for h2 in range(2):
                    # every head: [v(64)|one] -> out rows 0..64, D@64; the odd
                    # head's rows are DMA-shifted to yT rows 64..127 later.
                    geo.append(dict(
                        qrow=h2 * 64,
                        blk=(2 * pair + h2) * 65,
                        bw=65,
                        orow=0,
                    ))
                    o_halves.append([
                        psB.tile([128, 512], F32, tag="B", name=f"o{h2}_{half}")
                        for half in range(2)
                    ])
                q_tile = qkT_sb[pair]
                k_tile = qkT_sb[2 + pair]

                def extract_half(h2, half):
                    o_x = o_halves[h2][half]
                    c0 = i0 + half * 512
                    if h2 == 0:
                        nc.vector.tensor_copy(
                            yT_sb[pair][0:64, c0:c0 + 512], o_x[0:64, :]
                        )
                        nc.vector.reciprocal(
                            rw[64:65, c0:c0 + 512], o_x[64:65, :]
                        )
                    else:
                        # engines are lane-locked; DMA shifts partitions
                        # 0..63 -> yT rows 64..127.
                        nc.vector.tensor_copy(
                            stage[0:64, c0:c0 + 512], o_x[0:64, :]
                        )
                        nc.sync.dma_start(
                            yT_sb[pair][64:128, c0:c0 + 512],
                            stage[0:64, c0:c0 + 512],
                        )
                        nc.vector.reciprocal(
                            rwo[64:65, c0:c0 + 512], o_x[64:65, :]
                        )

                inject = list(inject)
                p_prev = None
                for jt in range(njt + 1):
                    p_cur = None
                    if jt < njt:
                        L = max(0, jt * 128 - i0)
                        w = 1024 - L
                        if w <= 512:
                            # narrow block: both heads share one psA tile
                            # (h0 in bank A at col 0, h1 in bank B at col 512)
                            # and a single strided exp covers both.
                            s_ps = psA.tile([128, 1024], F32, tag="A",
                                            name="s_ps")
                            for h2 in range(2):
                                g = geo[h2]
                                nc.tensor.matmul(
                                    s_ps[:, 512 * h2:512 * h2 + w],
                                    k_tile[g["qrow"]:g["qrow"] + 64,
                                           jt * 128:(jt + 1) * 128],
                                    q_tile[g["qrow"]:g["qrow"] + 64,
                                           i0 + L:i0 + L + w],
                                    start=True,
                                    stop=True,
                                )
                            p_sb = p_pool.tile([128, 1024], F16, tag="p",
                                               name="p_sb")
                            s3 = s_ps[:].rearrange("p (b c) -> p b c", b=2)
                            p3 = p_sb[:].rearrange("p (b c) -> p b c", b=2)
                            nc.scalar.activation(
                                p3[:, :, 0:w], s3[:, :, 0:w], AF.Exp,
                                scale=SCALE
                            )
                            if jt * 128 >= i0:  # diagonal: triangular mask
                                for h2 in range(2):
                                    nc.gpsimd.tensor_mul(
                                        p_sb[:, 512 * h2:512 * h2 + 128],
                                        p_sb[:, 512 * h2:512 * h2 + 128],
                                        mask_sb[:]
                                    )
                            p_cur = (p_sb, True)
                        else:
                            ps2 = []
                            for h2 in range(2):
                                g = geo[h2]
                                s_ps = psA.tile([128, 1024], F32, tag="A",
                                                name="s_ps")
                                for off, ww in _pieces(L):
                                    nc.tensor.matmul(
                                        s_ps[:, off:off + ww],
                                        k_tile[g["qrow"]:g["qrow"] + 64,
                                               jt * 128:(jt + 1) * 128],
                                        q_tile[g["qrow"]:g["qrow"] + 64,
                                               i0 + off:i0 + off + ww],
                                        start=True,
                                        stop=True,
                                    )
                                p_sb = p_pool.tile([128, 1024], F16, tag="p",
                                                   name="p_sb")
                                nc.scalar.activation(
                                    p_sb[:, L:1024], s_ps[:, L:1024], AF.Exp,
                                    scale=SCALE
                                )
                                if jt * 128 >= i0:  # diagonal: triangular mask
                                    nc.gpsimd.tensor_mul(
                                        p_sb[:, L:L + 128], p_sb[:, L:L + 128],
                                        mask_sb[:]
                                    )
                                ps2.append(p_sb)
                            p_cur = (ps2, False)
                    if inject and jt >= inj_start:
                        inject.pop(0)()
                    if jt > 0:
                        ja = jt - 1
                        La = max(0, ja * 128 - i0)
                        wa = 1024 - La
                        packed = p_prev[1]
                        for h2 in range(2):
                            g = geo[h2]
                            vap = v_sb[ja][:, g["blk"]:g["blk"] + g["bw"]]
                            if packed:
                                # p at col 512*h2, o cols [La-512, 512) bank B
                                nc.tensor.matmul(
                                    o_halves[h2][1][
                                        g["orow"]:g["orow"] + g["bw"],
                                        La - 512:512],
                                    vap,
                                    p_prev[0][:, 512 * h2:512 * h2 + wa],
                                    start=(ja == 0),
                                    stop=(ja == njt - 1),
                                    skip_group_check=True,
                                )
                            else:
                                for off, ww in _pieces(La):
                                    half = 0 if off < 512 else 1
                                    last = jtA_last if half == 0 else njt - 1
                                    nc.tensor.matmul(
                                        o_halves[h2][half][
                                            g["orow"]:g["orow"] + g["bw"],
                                            off - 512 * half:
                                            off - 512 * half + ww],
                                        vap,
                                        p_prev[0][h2][:, off:off + ww],
                                        start=(ja == 0),
                                        stop=(ja == last),
                                        skip_group_check=True,
                                    )
                            if ja == jtA_last:
                                extract_half(h2, 0)
                            if ja == njt - 1:
                                extract_half(h2, 1)
                    p_prev = p_cur
                for go in inject:  # flush any unconsumed injections
                    go()

            def emit_normalize(pair, ic):
                """GPSIMD-broadcast the pair's 1/D rows across partitions
                (even head -> rows 0..63, odd -> 64..127, matching the yT row
                split), then one full-width mul normalizes both heads' yT."""
                i0 = ic * 1024
                sl = slice(i0, i0 + 1024)
                rb_sb = rb_pool.tile([128, 1024], F32, tag="rb", name="rb_sb")
                scr = rbd[pair][ic]
                nc.gpsimd.dma_start(scr[0:1, :], rw[64:65, sl])
                nc.gpsimd.dma_start(scr[1:2, :], rwo[64:65, sl])
                nc.gpsimd.dma_start(
                    rb_sb[:], scr[:, :].unsqueeze(1).broadcast_to((2, 64, 1024))
                )
                yt = yT_sb[pair]
                nc.vector.tensor_mul(yt[:, sl], yt[:, sl], rb_sb[:])

            def proj_psA(tt):
                def go():
                    ps = psA.tile([128, 1024], F32, tag="A", name="pj_ps")
                    for mt in range(2):
                        for nch in range(2):
                            nc.tensor.matmul(
                                ps[:, nch * 512:(nch + 1) * 512],
                                yT_sb[mt][:, tt * 128:(tt + 1) * 128],
                                wp_sb[mt][:, nch * 512:(nch + 1) * 512],
                                start=(mt == 0),
                                stop=(mt == 1),
                            )
                    o_sb = o_pool.tile([128, 1024], F16, tag="o", name="o_sb")
                    nc.vector.tensor_copy(o_sb[:], ps[:, 0:1024])
                    nc.sync.dma_start(out[tt * 128:(tt + 1) * 128, :], o_sb[:])
                return go

            def emit_proj(tts):
                # proj runs on the psB ring ([128,512] halves), which is free
                # after attention — so the next rep's qkv can claim psA
                # without waiting for proj to drain.
                for tt in tts:
                    o_sb = o_pool.tile([128, 1024], F16, tag="o", name="o_sb")
                    for nch in range(2):
                        ps = psB.tile([128, 512], F32, tag="B", name="pj_ps")
                        for mt in range(2):
                            nc.tensor.matmul(
                                ps[:, 0:512],
                                yT_sb[mt][:, tt * 128:(tt + 1) * 128],
                                wp_sb[mt][:, nch * 512:(nch + 1) * 512],
                                start=(mt == 0),
                                stop=(mt == 1),
                            )
                        if (tt + nch) % 2 == 0:
                            nc.vector.tensor_copy(
                                o_sb[:, nch * 512:(nch + 1) * 512], ps[:, 0:512]
                            )
                        else:
                            nc.scalar.copy(
                                o_sb[:, nch * 512:(nch + 1) * 512], ps[:, 0:512]
                            )
                    eng = nc.sync if tt % 2 == 0 else nc.scalar
                    eng.dma_start(out[tt * 128:(tt + 1) * 128, :], o_sb[:])

            emit_qk((0, 2))        # q and k tiles for head pair A (h0, h1)
            for tt in range(8):    # v tiles read by the ic=0 key blocks
                v_group(tt, nc.scalar.copy)()
            # Attention is locally ACT(exp)-bound; feed the in-order PE with
            # later-needed independent matmul work inside each unit's jt loop.
            emit_attention_pair(
                pair := 0, 0, v_sb,
                inject=[v_group(tt, nc.vector.tensor_copy) for tt in range(8, 16)],
            )
            emit_normalize(0, 0)
            emit_attention_pair(
                0, 1, v_sb,
                inject=[qk_group(jt, ch) for ch in range(4) for jt in (1, 3)],
            )
            emit_normalize(0, 1)
            emit_attention_pair(1, 0, v_sb)
            emit_normalize(1, 0)
            emit_attention_pair(1, 1, v_sb)
            emit_normalize(1, 1)
            emit_proj(range(16))

    if not nc.is_finalized():
        nc.finalize()
    return nc


def host_prep(x, W_attn, b_attn, W_proj):
    f16 = np.float16
    x = np.ascontiguousarray(np.asarray(x, np.float32))
    W_attn = np.ascontiguousarray(np.asarray(W_attn, np.float32))
    b_attn = np.ascontiguousarray(np.asarray(b_attn, np.float32))
    W_proj = np.ascontiguousarray(np.asarray(W_proj, np.float32))
    mask = np.triu(np.ones((128, 128), f16))
    ones = np.ones((1, 128), f16)
    per_group = []
    for hg in range(NG):
        heads = [hg * HPG + i for i in range(HPG)]
        wq = np.concatenate([W_attn[:, h * HD:(h + 1) * HD] for h in heads], axis=1)
        wk = np.concatenate(
            [W_attn[:, C + h * HD:C + (h + 1) * HD] for h in heads], axis=1
        )
        wqk_ = np.ascontiguousarray(np.concatenate([wq, wk], axis=1).astype(f16))
        bq = np.concatenate([b_attn[h * HD:(h + 1) * HD] for h in heads])
        bk = np.concatenate([b_attn[C + h * HD:C + (h + 1) * HD] for h in heads])
        bqk_ = np.ascontiguousarray(np.concatenate([bq, bk]).reshape(4, 128).T)
        wv_ = np.zeros((C, VW), np.float32)
        bv_ = np.zeros((1, VW), np.float32)
        for i, h in enumerate(heads):
            v_off = i * 65
            one_off = i * 65 + 64
            wv_[:, v_off:v_off + 64] = \
                W_attn[:, 2 * C + h * HD:2 * C + (h + 1) * HD]
            bv_[0, v_off:v_off + 64] = \
                b_attn[2 * C + h * HD:2 * C + (h + 1) * HD]
            bv_[0, one_off] = 1.0
        wp_ = np.ascontiguousarray(
            np.concatenate(
                [W_proj[h * HD:(h + 1) * HD, :] for h in heads], axis=0
            ).astype(f16)
        )
        per_group.append((wqk_, bqk_, wv_.astype(f16), bv_.astype(f16), wp_))
    in_maps = []
    for b in range(B):
        xT_b = np.ascontiguousarray(x[b].T.astype(f16))
        for hg in range(NG):
            wqk_, bqk_, wv_, bv_, wp_ = per_group[hg]
            in_maps.append(
                dict(xT=xT_b, wqk=wqk_, bqk=bqk_, wv=wv_, bv=bv_, wp=wp_,
                     mask=mask, ones=ones)
            )
    return in_maps


_prog_cache = {}


def _get_program():
    if "nc" not in _prog_cache:
        _prog_cache["nc"] = build_program()
    return _prog_cache["nc"]


def run_cores(in_maps, trace=False, **kw):
    return run_bass_kernel_spmd(
        _get_program(), in_maps, list(range(NCORES)), trace=trace, **kw
    )


def kernel(x, W_attn, b_attn, W_proj, b_proj):
    in_maps = host_prep(x, W_attn, b_attn, W_proj)
    br = run_cores(in_maps)
    b_proj = np.asarray(b_proj, np.float32)
    y = np.zeros((B, T, C), np.float32)
    for b in range(B):
        acc = np.zeros((T, C), np.float32)
        for hg in range(NG):
            acc += np.asarray(br.results[b * NG + hg]["out"], np.float32)
        y[b] = acc + b_proj[None, :]
    return y
